# revision 6
# baseline (speedup 1.0000x reference)
"""Fused SwiGLU MLP (gate/up/down) Trainium2 Bass kernel.

Problem: y = down( silu(x @ Wg^T) * (x @ Wu^T) ) with
  x  [B=2, S=2048, H=4096]  f32
  Wg [I=11008, H]           f32   (gate proj, [out,in])
  Wu [I=11008, H]           f32
  Wd [H, I]                 f32

Strategy: data-parallel over tokens across the 8 NeuronCores.
Each core gets T = 4096/8 = 512 tokens and the full (replicated) weights,
computing the entire MLP for its token shard.  No collectives; the host
just concatenates the 8 token shards.  Per-core work: 138.6 GFLOP
(compute-bound: ~1.8 ms at the 78.6 TFLOP/s f32 PE roofline) vs ~532 MiB
of HBM traffic (~1.5 ms at ~360 GB/s), overlapped.

Device-side layout decisions (all transposes/tiling done on HOST in numpy
so every device DMA is a plain contiguous partition-major copy):
  x_host  [128, HS=32, T]          x^T tiled: [p, hs, t] = x[t, hs*128+p]
  wg_host [IC=22, 16, 128, 2, 512] Wg^T tiled (I padded 11008->11264)
  wu_host same
  wd_host [IC, 8, 128, 4, 512]     Wd^T tiled
  y out   [TT=4, 128, H]           y[tt*128+p, o]  (natural token-major)

Per-core kernel (per i-chunk ic of 512 padded-I columns):
  gate/up:  psum_g/u[it][128i, T] += Wg^T[h,i].T @ x^T[h,t]   (32 h-subtiles)
  mid:      hm[it] = silu(psum_g) * psum_u          (ACT + DVE)
  down:     psum_y[128t, 512o]    += hm[is][:,tt].T-as-lhsT @ Wd^T[i,o]
            y_sbuf[tt] += psum_y                    (DVE accumulate)
"""

import numpy as np

import concourse.bass as bass
import concourse.mybir as mybir
import concourse.tile as tile
from concourse import bacc
from concourse.bass_utils import run_bass_kernel_spmd

F32 = mybir.dt.float32
P = 128
ICW = 512  # i-chunk width (4 subtiles of 128)
OCW = 512  # o-chunk width

# full-size problem constants
B, S, H, I = 2, 2048, 4096, 11008
NCORES = 8
T = (B * S) // NCORES  # 512 tokens per core
IPAD = 11264           # 22 * 512


def build_nc(T, H, IPAD, wg_bufs=4, wd_bufs=4, hm_bufs=5, sg_bufs=2):
    assert T % P == 0 and T <= 512
    assert H % 512 == 0 and (H // P) % 2 == 0
    assert IPAD % ICW == 0
    HS = H // P       # h subtiles (contraction for gate/up)
    IC = IPAD // ICW  # i chunks
    NO = H // OCW     # o chunks
    TT = T // P       # token tiles

    nc = bacc.Bacc("TRN2", target_bir_lowering=False, debug=False)
    x_d = nc.dram_tensor("x", [P, HS, T], F32, kind="ExternalInput").ap()
    wg_d = nc.dram_tensor("wg", [IC, HS // 2, P, 2, ICW], F32, kind="ExternalInput").ap()
    wu_d = nc.dram_tensor("wu", [IC, HS // 2, P, 2, ICW], F32, kind="ExternalInput").ap()
    wd_d = nc.dram_tensor("wd", [IC, NO, P, ICW // P, OCW], F32, kind="ExternalInput").ap()
    y_d = nc.dram_tensor("y", [TT, P, H], F32, kind="ExternalOutput").ap()

    with tile.TileContext(nc) as tc:
        with (
            tc.tile_pool(name="xp", bufs=1) as xp,
            tc.tile_pool(name="yp", bufs=1) as yp,
            tc.tile_pool(name="wgp", bufs=wg_bufs) as wgp,
            tc.tile_pool(name="wup", bufs=wg_bufs) as wup,
            tc.tile_pool(name="wdp", bufs=wd_bufs) as wdp,
            tc.tile_pool(name="hmp", bufs=hm_bufs) as hmp,
            tc.tile_pool(name="sgp", bufs=sg_bufs) as sgp,
            tc.tile_pool(name="ps", bufs=8, space="PSUM") as ps,
        ):
            # resident x^T (8 MiB) and y accumulator (8 MiB)
            xt = xp.tile([P, HS, T], F32)
            nc.sync.dma_start(out=xt, in_=x_d)
            yt = []
            for tt in range(TT):
                ytile = yp.tile([P, H], F32, name=f"y{tt}", tag=f"y{tt}")
                nc.vector.memset(ytile, 0.0)
                yt.append(ytile)

            for ic in range(IC):
                # ---- gate/up projections, accumulated over all h ----
                psg = [ps.tile([P, T], F32, tag="ps", name=f"psg{k}") for k in range(4)]
                psu = [ps.tile([P, T], F32, tag="ps", name=f"psu{k}") for k in range(4)]
                for j in range(HS // 2):
                    gt = wgp.tile([P, 2, ICW], F32, tag="wg")
                    nc.sync.dma_start(out=gt, in_=wg_d[ic, j])
                    ut = wup.tile([P, 2, ICW], F32, tag="wu")
                    nc.sync.dma_start(out=ut, in_=wu_d[ic, j])
                    for h2 in range(2):
                        hs = 2 * j + h2
                        first, last = hs == 0, hs == HS - 1
                        for it in range(4):
                            nc.tensor.matmul(
                                psg[it],
                                gt[:, h2, it * P:(it + 1) * P],
                                xt[:, hs, :],
                                start=first, stop=last,
                            )
                        for it in range(4):
                            nc.tensor.matmul(
                                psu[it],
                                ut[:, h2, it * P:(it + 1) * P],
                                xt[:, hs, :],
                                start=first, stop=last,
                            )
                # ---- silu(gate) * up -> hm tiles [i128, T] ----
                hms = []
                for it in range(4):
                    sg = sgp.tile([P, T], F32, tag="sg")
                    nc.scalar.activation(
                        sg, psg[it], mybir.ActivationFunctionType.Sigmoid
                    )
                    # silu(g)*u = g*sigmoid(g)*u, two DVE muls (one PSUM operand each)
                    nc.vector.tensor_mul(sg, sg, psg[it])
                    hm = hmp.tile([P, T], F32, tag="hm")
                    nc.vector.tensor_mul(hm, sg, psu[it])
                    hms.append(hm)
                # ---- down projection for this i-chunk ----
                ISUB = ICW // P
                for osc in range(NO):
                    # wd for this (ic, osc) in two half tiles to keep SBUF slim
                    wdts = []
                    for half in range(2):
                        wdt = wdp.tile([P, ISUB // 2, OCW], F32, tag="wd", name=f"wd{half}")
                        nc.sync.dma_start(
                            out=wdt,
                            in_=wd_d[ic, osc, :, half * (ISUB // 2):(half + 1) * (ISUB // 2), :],
                        )
                        wdts.append(wdt)
                    for tt in range(TT):
                        py = ps.tile([P, OCW], F32, tag="ps", name="py")
                        for isub in range(ISUB):
                            nc.tensor.matmul(
                                py,
                                hms[isub][:, tt * P:(tt + 1) * P],
                                wdts[isub // (ISUB // 2)][:, isub % (ISUB // 2), :],
                                start=(isub == 0), stop=(isub == ISUB - 1),
                            )
                        osl = slice(osc * OCW, (osc + 1) * OCW)
                        nc.vector.tensor_add(yt[tt][:, osl], yt[tt][:, osl], py)

            for tt in range(TT):
                nc.sync.dma_start(out=y_d[tt], in_=yt[tt])

    nc.compile()
    return nc


def prep_weights(Wg, Wu, Wd, IPAD):
    """Host-side re-tiling of the weights into the device DMA layouts."""
    Iin, Hh = Wg.shape
    HS = Hh // P
    IC = IPAD // ICW
    NO = Hh // OCW
    f32 = np.float32

    Wg_p = np.zeros((IPAD, Hh), f32)
    Wg_p[:Iin] = Wg
    Wu_p = np.zeros((IPAD, Hh), f32)
    Wu_p[:Iin] = Wu
    Wd_p = np.zeros((Hh, IPAD), f32)
    Wd_p[:, :Iin] = Wd

    # wg[ic, j, p, h2, ii] = Wg_p[ic*ICW + ii, (2j+h2)*128 + p]
    wg_host = np.ascontiguousarray(
        Wg_p.reshape(IC, ICW, HS // 2, 2, P).transpose(0, 2, 4, 3, 1)
    )
    wu_host = np.ascontiguousarray(
        Wu_p.reshape(IC, ICW, HS // 2, 2, P).transpose(0, 2, 4, 3, 1)
    )
    # wd[ic, osc, p, isub, oo] = Wd_p[osc*OCW + oo, ic*ICW + isub*128 + p]
    wd_host = np.ascontiguousarray(
        Wd_p.reshape(NO, OCW, IC, ICW // P, P).transpose(2, 0, 4, 3, 1)
    )
    return wg_host, wu_host, wd_host


def prep_x_shard(x2, c, T):
    """x2 [tokens, H] -> core c's [128, HS, T] tile layout."""
    Hh = x2.shape[1]
    xs = x2[c * T:(c + 1) * T]  # [T, H]
    return np.ascontiguousarray(xs.reshape(T, Hh // P, P).transpose(2, 1, 0))


def run_on_cores(nc, in_maps, **kwargs):
    return run_bass_kernel_spmd(nc, in_maps, core_ids=list(range(len(in_maps))), **kwargs)


_NC_CACHE = {}


def _get_nc():
    key = (T, H, IPAD)
    if key not in _NC_CACHE:
        _NC_CACHE[key] = build_nc(T, H, IPAD)
    return _NC_CACHE[key]


def kernel(x, Wg, Wu, Wd, _trace=False, _trace_kwargs=None):
    x = np.asarray(x, np.float32)
    Wg = np.asarray(Wg, np.float32)
    Wu = np.asarray(Wu, np.float32)
    Wd = np.asarray(Wd, np.float32)

    nc = _get_nc()
    wg_host, wu_host, wd_host = prep_weights(Wg, Wu, Wd, IPAD)
    x2 = x.reshape(B * S, H)
    in_maps = [
        {
            "x": prep_x_shard(x2, c, T),
            "wg": wg_host,
            "wu": wu_host,
            "wd": wd_host,
        }
        for c in range(NCORES)
    ]
    kwargs = {}
    if _trace:
        kwargs["trace"] = True
        kwargs.update(_trace_kwargs or {})
    res = run_on_cores(nc, in_maps, **kwargs)
    shards = [res.results[c]["y"].reshape(T, H) for c in range(NCORES)]
    y = np.concatenate(shards, axis=0).reshape(B, S, H)
    if _trace:
        return y, res
    return y


# revision 7
# speedup vs baseline: 3.3638x; 3.3638x over previous
"""Fused SwiGLU MLP (gate/up/down) Trainium2 Bass kernel.

Problem: y = down( silu(x @ Wg^T) * (x @ Wu^T) ) with
  x  [B=2, S=2048, H=4096]  f32
  Wg [I=11008, H]           f32   (gate proj, [out,in])
  Wu [I=11008, H]           f32
  Wd [H, I]                 f32

Strategy: data-parallel over tokens across the 8 NeuronCores.
Each core gets T = 4096/8 = 512 tokens and the full (replicated) weights,
computing the entire MLP for its token shard.  No collectives; the host
just concatenates the 8 token shards.  Per-core work: 138.6 GFLOP
(compute-bound: ~1.8 ms at the 78.6 TFLOP/s f32 PE roofline) vs ~532 MiB
of HBM traffic (~1.5 ms at ~360 GB/s), overlapped.

Device-side layout decisions (all transposes/tiling done on HOST in numpy
so every device DMA is a plain contiguous partition-major copy):
  x_host  [128, HS=32, T]          x^T tiled: [p, hs, t] = x[t, hs*128+p]
  wg_host [IC=22, 16, 128, 2, 512] Wg^T tiled (I padded 11008->11264)
  wu_host same
  wd_host [IC, 8, 128, 4, 512]     Wd^T tiled
  y out   [TT=4, 128, H]           y[tt*128+p, o]  (natural token-major)

Per-core kernel (per i-chunk ic of 512 padded-I columns):
  gate/up:  psum_g/u[it][128i, T] += Wg^T[h,i].T @ x^T[h,t]   (32 h-subtiles)
  mid:      hm[it] = silu(psum_g) * psum_u          (ACT + DVE)
  down:     psum_y[128t, 512o]    += hm[is][:,tt].T-as-lhsT @ Wd^T[i,o]
            y_sbuf[tt] += psum_y                    (DVE accumulate)
"""

import numpy as np

import concourse.bass as bass
import concourse.mybir as mybir
import concourse.tile as tile
from concourse import bacc
from concourse.bass_utils import run_bass_kernel_spmd

F32 = mybir.dt.float32
F32R = mybir.dt.float32r
P = 128
ICW = 512  # i-chunk width (4 subtiles of 128)
OCW = 512  # o-chunk width

# full-size problem constants
B, S, H, I = 2, 2048, 4096, 11008
NCORES = 8
T = (B * S) // NCORES  # 512 tokens per core
IPAD = 11264           # 22 * 512


def build_nc(T, H, IPAD, wg_bufs=4, wd_bufs=4, hm_bufs=5, sg_bufs=2, mm_dt=F32):
    assert T % P == 0 and T <= 512
    assert H % 512 == 0 and (H // P) % 2 == 0
    assert IPAD % ICW == 0
    HS = H // P       # h subtiles (contraction for gate/up)
    IC = IPAD // ICW  # i chunks
    NO = H // OCW     # o chunks
    TT = T // P       # token tiles

    nc = bacc.Bacc("TRN2", target_bir_lowering=False, debug=False)
    x_d = nc.dram_tensor("x", [P, HS, T], mm_dt, kind="ExternalInput").ap()
    wg_d = nc.dram_tensor("wg", [IC, HS // 2, P, 2, ICW], mm_dt, kind="ExternalInput").ap()
    wu_d = nc.dram_tensor("wu", [IC, HS // 2, P, 2, ICW], mm_dt, kind="ExternalInput").ap()
    wd_d = nc.dram_tensor("wd", [IC, NO, P, ICW // P, OCW], mm_dt, kind="ExternalInput").ap()
    y_d = nc.dram_tensor("y", [TT, P, H], F32, kind="ExternalOutput").ap()

    with tile.TileContext(nc) as tc:
        with (
            tc.tile_pool(name="xp", bufs=1) as xp,
            tc.tile_pool(name="yp", bufs=1) as yp,
            tc.tile_pool(name="wgp", bufs=wg_bufs) as wgp,
            tc.tile_pool(name="wup", bufs=wg_bufs) as wup,
            tc.tile_pool(name="wdp", bufs=wd_bufs) as wdp,
            tc.tile_pool(name="hmp", bufs=hm_bufs) as hmp,
            tc.tile_pool(name="sgp", bufs=sg_bufs) as sgp,
            tc.tile_pool(name="ps", bufs=8, space="PSUM") as ps,
        ):
            # resident x^T (8 MiB) and y accumulator (8 MiB)
            xt = xp.tile([P, HS, T], mm_dt)
            nc.sync.dma_start(out=xt, in_=x_d)
            yt = []
            for tt in range(TT):
                ytile = yp.tile([P, H], F32, name=f"y{tt}", tag=f"y{tt}")
                nc.vector.memset(ytile, 0.0)
                yt.append(ytile)

            for ic in range(IC):
                # ---- gate/up projections, accumulated over all h ----
                psg = [ps.tile([P, T], F32, tag="ps", name=f"psg{k}") for k in range(4)]
                psu = [ps.tile([P, T], F32, tag="ps", name=f"psu{k}") for k in range(4)]
                for j in range(HS // 2):
                    gt = wgp.tile([P, 2, ICW], mm_dt, tag="wg")
                    nc.sync.dma_start(out=gt, in_=wg_d[ic, j])
                    ut = wup.tile([P, 2, ICW], mm_dt, tag="wu")
                    nc.sync.dma_start(out=ut, in_=wu_d[ic, j])
                    for h2 in range(2):
                        hs = 2 * j + h2
                        first, last = hs == 0, hs == HS - 1
                        for it in range(4):
                            nc.tensor.matmul(
                                psg[it],
                                gt[:, h2, it * P:(it + 1) * P],
                                xt[:, hs, :],
                                start=first, stop=last,
                            )
                        for it in range(4):
                            nc.tensor.matmul(
                                psu[it],
                                ut[:, h2, it * P:(it + 1) * P],
                                xt[:, hs, :],
                                start=first, stop=last,
                            )
                # ---- silu(gate) * up -> hm tiles [i128, T] ----
                hms = []
                for it in range(4):
                    sg = sgp.tile([P, T], F32, tag="sg")
                    nc.scalar.activation(
                        sg, psg[it], mybir.ActivationFunctionType.Sigmoid
                    )
                    # silu(g)*u = g*sigmoid(g)*u, two DVE muls (one PSUM operand each)
                    nc.vector.tensor_mul(sg, sg, psg[it])
                    hm = hmp.tile([P, T], mm_dt, tag="hm")
                    nc.vector.tensor_mul(hm, sg, psu[it])
                    hms.append(hm)
                # ---- down projection for this i-chunk ----
                ISUB = ICW // P
                for osc in range(NO):
                    # wd for this (ic, osc) in two half tiles to keep SBUF slim
                    wdts = []
                    for half in range(2):
                        wdt = wdp.tile([P, ISUB // 2, OCW], mm_dt, tag="wd", name=f"wd{half}")
                        nc.sync.dma_start(
                            out=wdt,
                            in_=wd_d[ic, osc, :, half * (ISUB // 2):(half + 1) * (ISUB // 2), :],
                        )
                        wdts.append(wdt)
                    for tt in range(TT):
                        py = ps.tile([P, OCW], F32, tag="ps", name="py")
                        for isub in range(ISUB):
                            nc.tensor.matmul(
                                py,
                                hms[isub][:, tt * P:(tt + 1) * P],
                                wdts[isub // (ISUB // 2)][:, isub % (ISUB // 2), :],
                                start=(isub == 0), stop=(isub == ISUB - 1),
                            )
                        osl = slice(osc * OCW, (osc + 1) * OCW)
                        nc.vector.tensor_add(yt[tt][:, osl], yt[tt][:, osl], py)

            for tt in range(TT):
                nc.sync.dma_start(out=y_d[tt], in_=yt[tt])

    nc.compile()
    return nc


def prep_weights(Wg, Wu, Wd, IPAD):
    """Host-side re-tiling of the weights into the device DMA layouts."""
    Iin, Hh = Wg.shape
    HS = Hh // P
    IC = IPAD // ICW
    NO = Hh // OCW
    f32 = np.float32

    Wg_p = np.zeros((IPAD, Hh), f32)
    Wg_p[:Iin] = Wg
    Wu_p = np.zeros((IPAD, Hh), f32)
    Wu_p[:Iin] = Wu
    Wd_p = np.zeros((Hh, IPAD), f32)
    Wd_p[:, :Iin] = Wd

    # wg[ic, j, p, h2, ii] = Wg_p[ic*ICW + ii, (2j+h2)*128 + p]
    wg_host = np.ascontiguousarray(
        Wg_p.reshape(IC, ICW, HS // 2, 2, P).transpose(0, 2, 4, 3, 1)
    )
    wu_host = np.ascontiguousarray(
        Wu_p.reshape(IC, ICW, HS // 2, 2, P).transpose(0, 2, 4, 3, 1)
    )
    # wd[ic, osc, p, isub, oo] = Wd_p[osc*OCW + oo, ic*ICW + isub*128 + p]
    wd_host = np.ascontiguousarray(
        Wd_p.reshape(NO, OCW, IC, ICW // P, P).transpose(2, 0, 4, 3, 1)
    )
    return wg_host, wu_host, wd_host


def prep_x_shard(x2, c, T):
    """x2 [tokens, H] -> core c's [128, HS, T] tile layout."""
    Hh = x2.shape[1]
    xs = x2[c * T:(c + 1) * T]  # [T, H]
    return np.ascontiguousarray(xs.reshape(T, Hh // P, P).transpose(2, 1, 0))


def run_on_cores(nc, in_maps, **kwargs):
    return run_bass_kernel_spmd(nc, in_maps, core_ids=list(range(len(in_maps))), **kwargs)


_NC_CACHE = {}

# matmul dtype mode: "f32" (exact, 4 PE cycles/row) or "f32r" (tf32-like,
# 1 PE cycle/row, ~2e-4 rel err)
MM_MODE = "f32r"


def _get_nc(mode=None):
    mode = mode or MM_MODE
    key = (T, H, IPAD, mode)
    if key not in _NC_CACHE:
        _NC_CACHE[key] = build_nc(T, H, IPAD, mm_dt=(F32R if mode == "f32r" else F32))
    return _NC_CACHE[key]


def kernel(x, Wg, Wu, Wd, _trace=False, _trace_kwargs=None, _mode=None):
    x = np.asarray(x, np.float32)
    Wg = np.asarray(Wg, np.float32)
    Wu = np.asarray(Wu, np.float32)
    Wd = np.asarray(Wd, np.float32)

    nc = _get_nc(_mode)
    wg_host, wu_host, wd_host = prep_weights(Wg, Wu, Wd, IPAD)
    x2 = x.reshape(B * S, H)
    in_maps = [
        {
            "x": prep_x_shard(x2, c, T),
            "wg": wg_host,
            "wu": wu_host,
            "wd": wd_host,
        }
        for c in range(NCORES)
    ]
    kwargs = {}
    if _trace:
        kwargs["trace"] = True
        kwargs.update(_trace_kwargs or {})
    res = run_on_cores(nc, in_maps, **kwargs)
    shards = [res.results[c]["y"].reshape(T, H) for c in range(NCORES)]
    y = np.concatenate(shards, axis=0).reshape(B, S, H)
    if _trace:
        return y, res
    return y


# revision 8
# speedup vs baseline: 3.4023x; 1.0114x over previous
"""Fused SwiGLU MLP (gate/up/down) Trainium2 Bass kernel.

Problem: y = down( silu(x @ Wg^T) * (x @ Wu^T) ) with
  x  [B=2, S=2048, H=4096]  f32
  Wg [I=11008, H]           f32   (gate proj, [out,in])
  Wu [I=11008, H]           f32
  Wd [H, I]                 f32

Strategy: data-parallel over tokens across the 8 NeuronCores.
Each core gets T = 4096/8 = 512 tokens and the full (replicated) weights,
computing the entire MLP for its token shard.  No collectives; the host
just concatenates the 8 token shards.  Per-core work: 138.6 GFLOP
(compute-bound: ~1.8 ms at the 78.6 TFLOP/s f32 PE roofline) vs ~532 MiB
of HBM traffic (~1.5 ms at ~360 GB/s), overlapped.

Device-side layout decisions (all transposes/tiling done on HOST in numpy
so every device DMA is a plain contiguous partition-major copy):
  x_host  [128, HS=32, T]          x^T tiled: [p, hs, t] = x[t, hs*128+p]
  wg_host [IC=22, 16, 128, 2, 512] Wg^T tiled (I padded 11008->11264)
  wu_host same
  wd_host [IC, 8, 128, 4, 512]     Wd^T tiled
  y out   [TT=4, 128, H]           y[tt*128+p, o]  (natural token-major)

Per-core kernel (per i-chunk ic of 512 padded-I columns):
  gate/up:  psum_g/u[it][128i, T] += Wg^T[h,i].T @ x^T[h,t]   (32 h-subtiles)
  mid:      hm[it] = silu(psum_g) * psum_u          (ACT + DVE)
  down:     psum_y[128t, 512o]    += hm[is][:,tt].T-as-lhsT @ Wd^T[i,o]
            y_sbuf[tt] += psum_y                    (DVE accumulate)
"""

import numpy as np

import concourse.bass as bass
import concourse.mybir as mybir
import concourse.tile as tile
from concourse import bacc
from concourse.bass_utils import run_bass_kernel_spmd

F32 = mybir.dt.float32
F32R = mybir.dt.float32r
P = 128
ICW = 512  # i-chunk width (4 subtiles of 128)
OCW = 512  # o-chunk width

# full-size problem constants
B, S, H, I = 2, 2048, 4096, 11008
NCORES = 8
T = (B * S) // NCORES  # 512 tokens per core
IPAD = 11264           # 22 * 512


def build_nc(T, H, IPAD, wg_bufs=6, wd_bufs=4, hm_bufs=5, sg_bufs=2, mm_dt=F32):
    assert T % P == 0 and T <= 512
    assert H % 512 == 0 and (H // P) % 2 == 0
    assert IPAD % ICW == 0
    HS = H // P       # h subtiles (contraction for gate/up)
    IC = IPAD // ICW  # i chunks
    NO = H // OCW     # o chunks
    TT = T // P       # token tiles

    nc = bacc.Bacc("TRN2", target_bir_lowering=False, debug=False)
    x_d = nc.dram_tensor("x", [P, HS, T], mm_dt, kind="ExternalInput").ap()
    wg_d = nc.dram_tensor("wg", [IC, HS // 2, P, 2, ICW], mm_dt, kind="ExternalInput").ap()
    wu_d = nc.dram_tensor("wu", [IC, HS // 2, P, 2, ICW], mm_dt, kind="ExternalInput").ap()
    wd_d = nc.dram_tensor("wd", [IC, NO, P, ICW // P, OCW], mm_dt, kind="ExternalInput").ap()
    y_d = nc.dram_tensor("y", [TT, P, H], F32, kind="ExternalOutput").ap()

    with tile.TileContext(nc) as tc:
        with (
            tc.tile_pool(name="xp", bufs=1) as xp,
            tc.tile_pool(name="yp", bufs=1) as yp,
            tc.tile_pool(name="wgp", bufs=wg_bufs) as wgp,
            tc.tile_pool(name="wup", bufs=wg_bufs) as wup,
            tc.tile_pool(name="wdp", bufs=wd_bufs) as wdp,
            tc.tile_pool(name="hmp", bufs=hm_bufs) as hmp,
            tc.tile_pool(name="sgp", bufs=sg_bufs) as sgp,
            tc.tile_pool(name="ps", bufs=8, space="PSUM") as ps,
        ):
            # resident x^T (8 MiB) and y accumulator (8 MiB)
            xt = xp.tile([P, HS, T], mm_dt)
            nc.sync.dma_start(out=xt, in_=x_d)
            yt = []
            for tt in range(TT):
                ytile = yp.tile([P, H], F32, name=f"y{tt}", tag=f"y{tt}")
                nc.vector.memset(ytile, 0.0)
                yt.append(ytile)

            for ic in range(IC):
                # ---- gate/up projections, accumulated over all h ----
                psg = [ps.tile([P, T], F32, tag="ps", name=f"psg{k}") for k in range(4)]
                psu = [ps.tile([P, T], F32, tag="ps", name=f"psu{k}") for k in range(4)]
                for j in range(HS // 2):
                    gt = wgp.tile([P, 2, ICW], mm_dt, tag="wg")
                    nc.sync.dma_start(out=gt, in_=wg_d[ic, j])
                    ut = wup.tile([P, 2, ICW], mm_dt, tag="wu")
                    nc.sync.dma_start(out=ut, in_=wu_d[ic, j])
                    for h2 in range(2):
                        hs = 2 * j + h2
                        first, last = hs == 0, hs == HS - 1
                        for it in range(4):
                            nc.tensor.matmul(
                                psg[it],
                                gt[:, h2, it * P:(it + 1) * P],
                                xt[:, hs, :],
                                start=first, stop=last,
                            )
                        for it in range(4):
                            nc.tensor.matmul(
                                psu[it],
                                ut[:, h2, it * P:(it + 1) * P],
                                xt[:, hs, :],
                                start=first, stop=last,
                            )
                # ---- silu(gate) * up -> hm tiles [i128, T] ----
                hms = []
                for it in range(4):
                    sg = sgp.tile([P, T], F32, tag="sg")
                    nc.scalar.activation(
                        sg, psg[it], mybir.ActivationFunctionType.Sigmoid
                    )
                    # silu(g)*u = g*sigmoid(g)*u, two DVE muls (one PSUM operand each)
                    nc.vector.tensor_mul(sg, sg, psg[it])
                    hm = hmp.tile([P, T], mm_dt, tag="hm")
                    nc.vector.tensor_mul(hm, sg, psu[it])
                    hms.append(hm)
                # ---- down projection for this i-chunk ----
                ISUB = ICW // P
                for osc in range(NO):
                    # wd for this (ic, osc) in two half tiles to keep SBUF slim
                    wdts = []
                    for half in range(2):
                        wdt = wdp.tile([P, ISUB // 2, OCW], mm_dt, tag="wd", name=f"wd{half}")
                        nc.sync.dma_start(
                            out=wdt,
                            in_=wd_d[ic, osc, :, half * (ISUB // 2):(half + 1) * (ISUB // 2), :],
                        )
                        wdts.append(wdt)
                    for tt in range(TT):
                        py = ps.tile([P, OCW], F32, tag="ps", name="py")
                        for isub in range(ISUB):
                            nc.tensor.matmul(
                                py,
                                hms[isub][:, tt * P:(tt + 1) * P],
                                wdts[isub // (ISUB // 2)][:, isub % (ISUB // 2), :],
                                start=(isub == 0), stop=(isub == ISUB - 1),
                            )
                        osl = slice(osc * OCW, (osc + 1) * OCW)
                        nc.vector.tensor_add(yt[tt][:, osl], yt[tt][:, osl], py)

            for tt in range(TT):
                nc.sync.dma_start(out=y_d[tt], in_=yt[tt])

    nc.compile()
    return nc


def prep_weights(Wg, Wu, Wd, IPAD):
    """Host-side re-tiling of the weights into the device DMA layouts."""
    Iin, Hh = Wg.shape
    HS = Hh // P
    IC = IPAD // ICW
    NO = Hh // OCW
    f32 = np.float32

    Wg_p = np.zeros((IPAD, Hh), f32)
    Wg_p[:Iin] = Wg
    Wu_p = np.zeros((IPAD, Hh), f32)
    Wu_p[:Iin] = Wu
    Wd_p = np.zeros((Hh, IPAD), f32)
    Wd_p[:, :Iin] = Wd

    # wg[ic, j, p, h2, ii] = Wg_p[ic*ICW + ii, (2j+h2)*128 + p]
    wg_host = np.ascontiguousarray(
        Wg_p.reshape(IC, ICW, HS // 2, 2, P).transpose(0, 2, 4, 3, 1)
    )
    wu_host = np.ascontiguousarray(
        Wu_p.reshape(IC, ICW, HS // 2, 2, P).transpose(0, 2, 4, 3, 1)
    )
    # wd[ic, osc, p, isub, oo] = Wd_p[osc*OCW + oo, ic*ICW + isub*128 + p]
    wd_host = np.ascontiguousarray(
        Wd_p.reshape(NO, OCW, IC, ICW // P, P).transpose(2, 0, 4, 3, 1)
    )
    return wg_host, wu_host, wd_host


def prep_x_shard(x2, c, T):
    """x2 [tokens, H] -> core c's [128, HS, T] tile layout."""
    Hh = x2.shape[1]
    xs = x2[c * T:(c + 1) * T]  # [T, H]
    return np.ascontiguousarray(xs.reshape(T, Hh // P, P).transpose(2, 1, 0))


def run_on_cores(nc, in_maps, **kwargs):
    return run_bass_kernel_spmd(nc, in_maps, core_ids=list(range(len(in_maps))), **kwargs)


_NC_CACHE = {}

# matmul dtype mode: "f32" (exact, 4 PE cycles/row) or "f32r" (tf32-like,
# 1 PE cycle/row, ~2e-4 rel err)
MM_MODE = "f32r"


def _get_nc(mode=None):
    mode = mode or MM_MODE
    key = (T, H, IPAD, mode)
    if key not in _NC_CACHE:
        _NC_CACHE[key] = build_nc(T, H, IPAD, mm_dt=(F32R if mode == "f32r" else F32))
    return _NC_CACHE[key]


def kernel(x, Wg, Wu, Wd, _trace=False, _trace_kwargs=None, _mode=None):
    x = np.asarray(x, np.float32)
    Wg = np.asarray(Wg, np.float32)
    Wu = np.asarray(Wu, np.float32)
    Wd = np.asarray(Wd, np.float32)

    nc = _get_nc(_mode)
    wg_host, wu_host, wd_host = prep_weights(Wg, Wu, Wd, IPAD)
    x2 = x.reshape(B * S, H)
    in_maps = [
        {
            "x": prep_x_shard(x2, c, T),
            "wg": wg_host,
            "wu": wu_host,
            "wd": wd_host,
        }
        for c in range(NCORES)
    ]
    kwargs = {}
    if _trace:
        kwargs["trace"] = True
        kwargs.update(_trace_kwargs or {})
    res = run_on_cores(nc, in_maps, **kwargs)
    shards = [res.results[c]["y"].reshape(T, H) for c in range(NCORES)]
    y = np.concatenate(shards, axis=0).reshape(B, S, H)
    if _trace:
        return y, res
    return y
